# revision 8
# baseline (speedup 1.0000x reference)
"""nn_Attention_30511447671564 — Trainium2 Bass kernel.

Head-mixing attention block. Shapes (hardcoded): B=64, T=64, C=4096,
H=64, hd=64, rank=1.  For every token (b,t): attention mixes the 64
heads (HxH scores, causal over head index).

    qkv = x @ W_atten^T + b_atten                  (B,T,3C)
    per-token: s[i,j] = q_i . k_j / 8  (i,j heads, causal j<=i)
               att = softmax_j(s);  y_i = sum_j att[i,j] v_j
    out = y @ W_proj^T + b_proj                    (B,T,C)

Distribution: pure data-parallel — 8 cores x 512 tokens, no collectives.

Per-core device pipeline (all matmuls bf16 with fp32 PSUM accumulation):
  1. QKV "Form 2": qkvT[o, t] = sum_c WaT[c, o] * xT[c, t]; bias (and the
     1/8 score scale, folded into k) applied on PSUM eviction.
  2. Rotation: evicted feature tiles [(h,d) x t] are DMA'd to DRAM bounce
     tensors Qrot/Krot [d, h, t] and Vrot [h, d, t] (contiguous runs).
  3. Attention in 128-token chunks: per token, 64x64 matmuls
     sT = K_t^T-style scores with heads contracted over d; exp (no max
     subtraction — scores are O(10), safe in fp32); causal mask applied
     multiplicatively after exp; softmax denominators via a ones-column
     matmul off the same stationary attT; normalization folded into the
     PSUM eviction scale. y is written (d-major) and DMA'd to Ydram.
  4. Projection "Form 2" from Ydram, bias on eviction, transposed output
     outT[c, t] written to DRAM; host un-transposes.
"""

import numpy as np
import ml_dtypes
from contextlib import ExitStack

import concourse.bass as bass
import concourse.tile as tile
from concourse import bacc, mybir
from concourse.bass_utils import run_bass_kernel_spmd

F32 = mybir.dt.float32
BF16 = mybir.dt.bfloat16
ACT = mybir.ActivationFunctionType

N_CORES = 8
B, T, C = 64, 64, 4096
H, HD = 64, 64
NTOK = (B // N_CORES) * T            # 512 tokens per core
O3 = 3 * C                           # 12288
NM = O3 // 128                       # 96 feature tiles of 128
NK = C // 128                        # 32 contraction tiles of 128
TCH = 128                            # attention token chunk
NGRP = 8                             # tokens per attention group


def _build_program():
    nc = bacc.Bacc(
        "TRN2", target_bir_lowering=False, debug=False, num_devices=N_CORES
    )

    xT = nc.declare_dram_parameter("xT", [C, NTOK], BF16, isOutput=False)
    WaT = nc.declare_dram_parameter("WaT", [C, O3], BF16, isOutput=False)
    WpT = nc.declare_dram_parameter("WpT", [C, C], BF16, isOutput=False)
    b_att = nc.declare_dram_parameter("b_att", [128, NM], F32, isOutput=False)
    b_prj = nc.declare_dram_parameter("b_prj", [128, NK], F32, isOutput=False)
    maskT8 = nc.declare_dram_parameter("maskT8", [H, NGRP * H], BF16, isOutput=False)
    onesc = nc.declare_dram_parameter("onesc", [H, 1], BF16, isOutput=False)
    outT = nc.declare_dram_parameter("outT", [C, NTOK], F32, isOutput=True)

    with tile.TileContext(nc) as tc, ExitStack() as ctx:
        _emit(ctx, tc, xT, WaT, WpT, b_att, b_prj, maskT8, onesc, outT)
    nc.compile()
    return nc


def _emit(ctx, tc, xT, WaT, WpT, b_att, b_prj, maskT8, onesc, outT):
    nc = tc.nc

    const = ctx.enter_context(tc.tile_pool(name="const", bufs=1))
    b_att_sb = const.tile([128, NM], F32)
    nc.sync.dma_start(b_att_sb[:], b_att.ap())
    b_prj_sb = const.tile([128, NK], F32)
    nc.sync.dma_start(b_prj_sb[:], b_prj.ap())
    mask_sb = const.tile([H, NGRP * H], BF16)
    nc.sync.dma_start(mask_sb[:], maskT8.ap())
    ones_sb = const.tile([H, 1], BF16)
    nc.sync.dma_start(ones_sb[:], onesc.ap())

    dram = ctx.enter_context(tc.tile_pool(name="dram", bufs=1, space="DRAM"))
    Qrot = dram.tile([HD, H * NTOK], BF16)    # [d, (h, t)]
    Krot = dram.tile([HD, H * NTOK], BF16)    # [d, (h, t)]
    Vrot = dram.tile([H, HD * NTOK], BF16)    # [h, (d, t)]
    Ydram = dram.tile([H, HD * NTOK], BF16)   # [i, (d, t)] == yT row-major

    q3 = Qrot.rearrange("d (h t) -> d h t", t=NTOK)
    k3 = Krot.rearrange("d (h t) -> d h t", t=NTOK)
    v3 = Vrot.rearrange("h (d t) -> h d t", t=NTOK)
    y3 = Ydram.rearrange("i (d t) -> i d t", t=NTOK)

    # ---------------- Stage 1: QKV + rotation to DRAM ----------------
    wpool = ctx.enter_context(tc.tile_pool(name="wpool", bufs=3))
    ps1 = ctx.enter_context(tc.tile_pool(name="ps1", bufs=2, space="PSUM"))
    secp = ctx.enter_context(tc.tile_pool(name="secp", bufs=4))

    with tc.tile_pool(name="xpool", bufs=1) as xpool:
        x_sb = xpool.tile([128, NK * NTOK], BF16)
        nc.sync.dma_start(
            x_sb.rearrange("p (k t) -> p k t", t=NTOK),
            xT.ap().rearrange("(k p) t -> p k t", p=128),
        )

        for m in range(NM):
            wa = wpool.tile([128, NK * 128], BF16, name=f"wa{m}", tag="wa")
            nc.sync.dma_start(
                wa.rearrange("p (k o) -> p k o", o=128),
                WaT.ap()[:, m * 128 : (m + 1) * 128].rearrange(
                    "(k p) o -> p k o", p=128
                ),
            )
            ps = ps1.tile([128, NTOK], F32, name=f"ps{m}", tag="ps")
            for kc in range(NK):
                nc.tensor.matmul(
                    ps[:],
                    wa[:, kc * 128 : (kc + 1) * 128],
                    x_sb[:, kc * NTOK : (kc + 1) * NTOK],
                    start=(kc == 0),
                    stop=(kc == NK - 1),
                )
            sec = secp.tile([128, NTOK], BF16, name=f"sec{m}", tag="sec")
            is_k = NK <= m < 2 * NK
            nc.scalar.activation(
                sec[:],
                ps[:],
                ACT.Identity,
                bias=b_att_sb[:, m : m + 1],
                scale=0.125 if is_k else 1.0,
            )
            # rotate the two heads of this tile out to DRAM
            if m < NK:
                for h2 in range(2):
                    h = m * 2 + h2
                    nc.sync.dma_start(q3[:, h, :], sec[h2 * 64 : h2 * 64 + 64, :])
            elif m < 2 * NK:
                for h2 in range(2):
                    h = (m - NK) * 2 + h2
                    nc.sync.dma_start(k3[:, h, :], sec[h2 * 64 : h2 * 64 + 64, :])
            else:
                for h2 in range(2):
                    h = (m - 2 * NK) * 2 + h2
                    nc.sync.dma_start(v3[h, :, :], sec[h2 * 64 : h2 * 64 + 64, :])

    # ---------------- Stage 2: per-token head attention ----------------
    qkp = ctx.enter_context(tc.tile_pool(name="qkp", bufs=2))
    psA = ctx.enter_context(tc.tile_pool(name="psA", bufs=2, space="PSUM"))
    psB = ctx.enter_context(tc.tile_pool(name="psB", bufs=2, space="PSUM"))
    psC = ctx.enter_context(tc.tile_pool(name="psC", bufs=2, space="PSUM"))
    atp = ctx.enter_context(tc.tile_pool(name="atp", bufs=3))

    for c0 in range(0, NTOK, TCH):
        qr = qkp.tile([HD, H * TCH], BF16, name=f"qr{c0}", tag="qr")
        kr = qkp.tile([HD, H * TCH], BF16, name=f"kr{c0}", tag="kr")
        vr = qkp.tile([H, HD * TCH], BF16, name=f"vr{c0}", tag="vr")
        nc.sync.dma_start(
            qr.rearrange("d (h t) -> d h t", t=TCH), q3[:, :, c0 : c0 + TCH]
        )
        nc.sync.dma_start(
            kr.rearrange("d (h t) -> d h t", t=TCH), k3[:, :, c0 : c0 + TCH]
        )
        nc.sync.dma_start(
            vr.rearrange("h (d t) -> h d t", t=TCH), v3[:, :, c0 : c0 + TCH]
        )
        kr3 = kr.rearrange("d (h t) -> d h t", t=TCH)
        qr3 = qr.rearrange("d (h t) -> d h t", t=TCH)
        vr3 = vr.rearrange("h (d t) -> h d t", t=TCH)

        for g in range(TCH // NGRP):
            t0 = g * NGRP
            ps_s = psA.tile([H, NGRP * H], F32, name=f"pss{c0}_{g}", tag="pss")
            for tl in range(NGRP):
                t = t0 + tl
                nc.tensor.matmul(
                    ps_s[:, tl * H : (tl + 1) * H],
                    kr3[:, :, t],
                    qr3[:, :, t],
                    start=True,
                    stop=True,
                )
            exp_sb = atp.tile([H, NGRP * H], BF16, name=f"exp{c0}_{g}", tag="exp")
            nc.scalar.activation(exp_sb[:], ps_s[:], ACT.Exp)
            att = atp.tile([H, NGRP * H], BF16, name=f"att{c0}_{g}", tag="att")
            nc.vector.tensor_mul(att[:], exp_sb[:], mask_sb[:])

            ps_y = psB.tile([H, NGRP * H], F32, name=f"psy{c0}_{g}", tag="psy")
            ps_n = psC.tile([H, NGRP], F32, name=f"psn{c0}_{g}", tag="psn")
            for tl in range(NGRP):
                a_t = att[:, tl * H : (tl + 1) * H]
                nc.tensor.matmul(
                    ps_n[:, tl : tl + 1], a_t, ones_sb[:], start=True, stop=True
                )
                nc.tensor.matmul(
                    ps_y[:, tl * H : (tl + 1) * H],
                    a_t,
                    vr3[:, :, t0 + tl],
                    start=True,
                    stop=True,
                )
            rs = atp.tile([H, NGRP], F32, name=f"rs{c0}_{g}", tag="rs")
            nc.vector.reciprocal(rs[:], ps_n[:])
            y_st = atp.tile([H, HD * NGRP], BF16, name=f"yst{c0}_{g}", tag="yst")
            y_st3 = y_st.rearrange("i (d t) -> i d t", t=NGRP)
            for tl in range(NGRP):
                nc.scalar.activation(
                    y_st3[:, :, tl],
                    ps_y[:, tl * H : (tl + 1) * H],
                    ACT.Copy,
                    scale=rs[:, tl : tl + 1],
                )
            gt0 = c0 + t0
            nc.sync.dma_start(y3[:, :, gt0 : gt0 + NGRP], y_st3)

    # ---------------- Stage 3: projection ----------------
    outp = ctx.enter_context(tc.tile_pool(name="outp", bufs=3))
    with tc.tile_pool(name="ypool", bufs=1) as ypool:
        yt_sb = ypool.tile([128, NK * NTOK], BF16)
        for p2 in range(2):
            nc.sync.dma_start(
                yt_sb.rearrange("(p2 d) (k t) -> p2 d k t", p2=2, t=NTOK)[p2],
                Ydram.rearrange("(k p2) (d t) -> p2 d k t", p2=2, t=NTOK)[p2],
            )
        for mo in range(NK):
            wp = wpool.tile([128, NK * 128], BF16, name=f"wp{mo}", tag="wa")
            nc.sync.dma_start(
                wp.rearrange("p (k o) -> p k o", o=128),
                WpT.ap()[:, mo * 128 : (mo + 1) * 128].rearrange(
                    "(k p) o -> p k o", p=128
                ),
            )
            ps = ps1.tile([128, NTOK], F32, name=f"pso{mo}", tag="ps")
            for kc in range(NK):
                nc.tensor.matmul(
                    ps[:],
                    wp[:, kc * 128 : (kc + 1) * 128],
                    yt_sb[:, kc * NTOK : (kc + 1) * NTOK],
                    start=(kc == 0),
                    stop=(kc == NK - 1),
                )
            ob = outp.tile([128, NTOK], F32, name=f"ob{mo}", tag="ob")
            nc.scalar.activation(
                ob[:], ps[:], ACT.Identity, bias=b_prj_sb[:, mo : mo + 1]
            )
            nc.sync.dma_start(outT.ap()[mo * 128 : (mo + 1) * 128, :], ob[:])


_PROGRAM = None


def _get_program():
    global _PROGRAM
    if _PROGRAM is None:
        _PROGRAM = _build_program()
    return _PROGRAM


def _host_inputs(x, W_atten, b_atten, W_proj, b_proj):
    bf = ml_dtypes.bfloat16
    x = np.asarray(x, np.float32).reshape(B, T, C)
    WaT = np.ascontiguousarray(np.asarray(W_atten, np.float32).T).astype(bf)
    WpT = np.ascontiguousarray(np.asarray(W_proj, np.float32).T).astype(bf)
    ba = np.asarray(b_atten, np.float32).copy()
    ba[C : 2 * C] *= 0.125  # fold the 1/sqrt(hd) score scale into k bias
    b_att_h = np.ascontiguousarray(ba.reshape(NM, 128).T)
    b_prj_h = np.ascontiguousarray(
        np.asarray(b_proj, np.float32).reshape(NK, 128).T
    )
    mask = np.tril(np.ones((H, H), np.float32))  # keep j<=i
    maskT8_h = np.ascontiguousarray(np.tile(mask.T, (1, NGRP))).astype(bf)
    ones_h = np.ones((H, 1), bf)

    shard_b = B // N_CORES
    in_maps = []
    for i in range(N_CORES):
        xs = x[i * shard_b : (i + 1) * shard_b].reshape(NTOK, C)
        xT_i = np.ascontiguousarray(xs.T).astype(bf)
        in_maps.append(
            {
                "xT": xT_i,
                "WaT": WaT,
                "WpT": WpT,
                "b_att": b_att_h,
                "b_prj": b_prj_h,
                "maskT8": maskT8_h,
                "onesc": ones_h,
            }
        )
    return in_maps


def run(inputs, trace=False):
    nc = _get_program()
    in_maps = _host_inputs(**inputs)
    res = run_bass_kernel_spmd(
        nc, in_maps, list(range(N_CORES)), trace=trace
    )
    shard_b = B // N_CORES
    out = np.empty((B, T, C), dtype=np.float32)
    for i in range(N_CORES):
        oT = np.asarray(res.results[i]["outT"], np.float32)  # (C, NTOK)
        out[i * shard_b : (i + 1) * shard_b] = oT.T.reshape(shard_b, T, C)
    return out, res


def kernel(x, W_atten, b_atten, W_proj, b_proj):
    out, _ = run(
        dict(
            x=x,
            W_atten=W_atten,
            b_atten=b_atten,
            W_proj=W_proj,
            b_proj=b_proj,
        )
    )
    return out


# revision 12
# speedup vs baseline: 54.2801x; 54.2801x over previous
"""nn_Attention_30511447671564 — Trainium2 Bass kernel.

Head-mixing attention block. Shapes (hardcoded): B=64, T=64, C=4096,
H=64, hd=64, rank=1.  For every token (b,t): attention mixes the 64
heads (HxH scores, causal over head index).

    qkv = x @ W_atten^T + b_atten                  (B,T,3C)
    per-token: s[i,j] = q_i . k_j / 8  (i,j heads, causal j<=i)
               att = softmax_j(s);  y_i = sum_j att[i,j] v_j
    out = y @ W_proj^T + b_proj                    (B,T,C)

Distribution: pure data-parallel — 8 cores x 512 tokens, no collectives.

Per-core device pipeline (all matmuls bf16 with fp32 PSUM accumulation):
  1. QKV "Form 2": qkvT[o, t] = sum_c WaT[c, o] * xT[c, t]; bias (and the
     1/8 score scale, folded into k) applied on PSUM eviction.
  2. Rotation: evicted feature tiles [(h,d) x t] are DMA'd to DRAM bounce
     tensors Qrot/Krot [d, h, t] and Vrot [h, d, t] (contiguous runs).
  3. Attention in 128-token chunks: per token, 64x64 matmuls
     sT = K_t^T-style scores with heads contracted over d; exp (no max
     subtraction — scores are O(10), safe in fp32); causal mask applied
     multiplicatively after exp; softmax denominators via a ones-column
     matmul off the same stationary attT; normalization folded into the
     PSUM eviction scale. y is written (d-major) and DMA'd to Ydram.
  4. Projection "Form 2" from Ydram, bias on eviction, transposed output
     outT[c, t] written to DRAM; host un-transposes.
"""

import numpy as np
import ml_dtypes
from contextlib import ExitStack

import concourse.bass as bass
import concourse.tile as tile
from concourse import bacc, mybir
from concourse.bass_utils import run_bass_kernel_spmd

F32 = mybir.dt.float32
BF16 = mybir.dt.bfloat16
ACT = mybir.ActivationFunctionType

N_CORES = 8
B, T, C = 64, 64, 4096
H, HD = 64, 64
NTOK = (B // N_CORES) * T            # 512 tokens per core
O3 = 3 * C                           # 12288
NM = O3 // 128                       # 96 feature tiles of 128
NK = C // 128                        # 32 contraction tiles of 128
TCH = 128                            # attention token chunk
NGRP = 8                             # tokens per attention group


def _build_program(reps=1):
    nc = bacc.Bacc(
        "TRN2", target_bir_lowering=False, debug=False, num_devices=N_CORES
    )

    xT = nc.declare_dram_parameter("xT", [C, NTOK], BF16, isOutput=False)
    WaT = nc.declare_dram_parameter("WaT", [C, O3], BF16, isOutput=False)
    WpT = nc.declare_dram_parameter("WpT", [C, C], BF16, isOutput=False)
    b_att = nc.declare_dram_parameter("b_att", [128, NM], F32, isOutput=False)
    b_prj = nc.declare_dram_parameter("b_prj", [128, NK], F32, isOutput=False)
    maskT8 = nc.declare_dram_parameter("maskT8", [H, NGRP * H], BF16, isOutput=False)
    onesc = nc.declare_dram_parameter("onesc", [H, 1], BF16, isOutput=False)
    outT = nc.declare_dram_parameter("outT", [C, NTOK], F32, isOutput=True)

    with tile.TileContext(nc) as tc, ExitStack() as ctx:
        for rep in range(reps):
            with ExitStack() as rctx:
                _emit(
                    rctx, tc, xT, WaT, WpT, b_att, b_prj, maskT8, onesc,
                    outT, rep,
                )
    nc.compile()
    return nc


def _emit(ctx, tc, xT, WaT, WpT, b_att, b_prj, maskT8, onesc, outT, rep=0):
    nc = tc.nc
    R = f"r{rep}_"

    const = ctx.enter_context(tc.tile_pool(name=R + "const", bufs=1))
    b_att_sb = const.tile([128, NM], F32)
    nc.sync.dma_start(b_att_sb[:], b_att.ap())
    b_prj_sb = const.tile([128, NK], F32)
    nc.sync.dma_start(b_prj_sb[:], b_prj.ap())
    mask_sb = const.tile([H, NGRP * H], BF16)
    nc.sync.dma_start(mask_sb[:], maskT8.ap())
    ones_sb = const.tile([H, 1], BF16)
    nc.sync.dma_start(ones_sb[:], onesc.ap())

    dram = ctx.enter_context(tc.tile_pool(name=R + "dram", bufs=1, space="DRAM"))
    Qrot = dram.tile([HD, H * NTOK], BF16)    # [d, (h, t)]
    Krot = dram.tile([HD, H * NTOK], BF16)    # [d, (h, t)]
    Vrot = dram.tile([H, HD * NTOK], BF16)    # [h, (d, t)]
    Ydram = dram.tile([H, HD * NTOK], BF16)   # [i, (d, t)] == yT row-major

    q3 = Qrot.rearrange("d (h t) -> d h t", t=NTOK)
    k3 = Krot.rearrange("d (h t) -> d h t", t=NTOK)
    v3 = Vrot.rearrange("h (d t) -> h d t", t=NTOK)
    y3 = Ydram.rearrange("i (d t) -> i d t", t=NTOK)

    # ---------------- Stage 1: QKV + rotation to DRAM ----------------
    wpool = ctx.enter_context(tc.tile_pool(name=R + "wpool", bufs=3))
    ps1 = ctx.enter_context(tc.tile_pool(name=R + "ps1", bufs=2, space="PSUM"))
    secp = ctx.enter_context(tc.tile_pool(name=R + "secp", bufs=4))

    with tc.tile_pool(name=R + "xpool", bufs=1) as xpool:
        x_sb = xpool.tile([128, NK * NTOK], BF16)
        nc.sync.dma_start(
            x_sb.rearrange("p (k t) -> p k t", t=NTOK),
            xT.ap().rearrange("(k p) t -> p k t", p=128),
        )

        for m in range(NM):
            wa = wpool.tile([128, NK * 128], BF16, name=f"{R}wa{m}", tag="wa")
            nc.sync.dma_start(
                wa.rearrange("p (k o) -> p k o", o=128),
                WaT.ap()[:, m * 128 : (m + 1) * 128].rearrange(
                    "(k p) o -> p k o", p=128
                ),
            )
            ps = ps1.tile([128, NTOK], F32, name=f"{R}ps{m}", tag="ps")
            for kc in range(NK):
                nc.tensor.matmul(
                    ps[:],
                    wa[:, kc * 128 : (kc + 1) * 128],
                    x_sb[:, kc * NTOK : (kc + 1) * NTOK],
                    start=(kc == 0),
                    stop=(kc == NK - 1),
                )
            sec = secp.tile([128, NTOK], BF16, name=f"{R}sec{m}", tag="sec")
            is_k = NK <= m < 2 * NK
            nc.scalar.activation(
                sec[:],
                ps[:],
                ACT.Identity,
                bias=b_att_sb[:, m : m + 1],
                scale=0.125 if is_k else 1.0,
            )
            # rotate the two heads of this tile out to DRAM
            if m < NK:
                for h2 in range(2):
                    h = m * 2 + h2
                    nc.sync.dma_start(q3[:, h, :], sec[h2 * 64 : h2 * 64 + 64, :])
            elif m < 2 * NK:
                for h2 in range(2):
                    h = (m - NK) * 2 + h2
                    nc.sync.dma_start(k3[:, h, :], sec[h2 * 64 : h2 * 64 + 64, :])
            else:
                for h2 in range(2):
                    h = (m - 2 * NK) * 2 + h2
                    nc.sync.dma_start(v3[h, :, :], sec[h2 * 64 : h2 * 64 + 64, :])

    # ---------------- Stage 2: per-token head attention ----------------
    qkp = ctx.enter_context(tc.tile_pool(name=R + "qkp", bufs=2))
    psA = ctx.enter_context(tc.tile_pool(name=R + "psA", bufs=2, space="PSUM"))
    psB = ctx.enter_context(tc.tile_pool(name=R + "psB", bufs=2, space="PSUM"))
    psC = ctx.enter_context(tc.tile_pool(name=R + "psC", bufs=2, space="PSUM"))
    atp = ctx.enter_context(tc.tile_pool(name=R + "atp", bufs=3))

    for c0 in range(0, NTOK, TCH):
        qr = qkp.tile([HD, H * TCH], BF16, name=f"{R}qr{c0}", tag="qr")
        kr = qkp.tile([HD, H * TCH], BF16, name=f"{R}kr{c0}", tag="kr")
        vr = qkp.tile([H, HD * TCH], BF16, name=f"{R}vr{c0}", tag="vr")
        nc.sync.dma_start(
            qr.rearrange("d (h t) -> d h t", t=TCH), q3[:, :, c0 : c0 + TCH]
        )
        nc.sync.dma_start(
            kr.rearrange("d (h t) -> d h t", t=TCH), k3[:, :, c0 : c0 + TCH]
        )
        nc.sync.dma_start(
            vr.rearrange("h (d t) -> h d t", t=TCH), v3[:, :, c0 : c0 + TCH]
        )
        kr3 = kr.rearrange("d (h t) -> d h t", t=TCH)
        qr3 = qr.rearrange("d (h t) -> d h t", t=TCH)
        vr3 = vr.rearrange("h (d t) -> h d t", t=TCH)

        for g in range(TCH // NGRP):
            t0 = g * NGRP
            ps_s = psA.tile([H, NGRP * H], F32, name=f"{R}pss{c0}_{g}", tag="pss")
            for tl in range(NGRP):
                t = t0 + tl
                nc.tensor.matmul(
                    ps_s[:, tl * H : (tl + 1) * H],
                    kr3[:, :, t],
                    qr3[:, :, t],
                    start=True,
                    stop=True,
                )
            exp_sb = atp.tile([H, NGRP * H], BF16, name=f"{R}exp{c0}_{g}", tag="exp")
            nc.scalar.activation(exp_sb[:], ps_s[:], ACT.Exp)
            att = atp.tile([H, NGRP * H], BF16, name=f"{R}att{c0}_{g}", tag="att")
            nc.vector.tensor_mul(att[:], exp_sb[:], mask_sb[:])

            ps_y = psB.tile([H, NGRP * H], F32, name=f"{R}psy{c0}_{g}", tag="psy")
            ps_n = psC.tile([H, NGRP], F32, name=f"{R}psn{c0}_{g}", tag="psn")
            for tl in range(NGRP):
                a_t = att[:, tl * H : (tl + 1) * H]
                nc.tensor.matmul(
                    ps_n[:, tl : tl + 1], a_t, ones_sb[:], start=True, stop=True
                )
                nc.tensor.matmul(
                    ps_y[:, tl * H : (tl + 1) * H],
                    a_t,
                    vr3[:, :, t0 + tl],
                    start=True,
                    stop=True,
                )
            rs = atp.tile([H, NGRP], F32, name=f"{R}rs{c0}_{g}", tag="rs")
            nc.vector.reciprocal(rs[:], ps_n[:])
            y_st = atp.tile([H, HD * NGRP], BF16, name=f"{R}yst{c0}_{g}", tag="yst")
            y_st3 = y_st.rearrange("i (d t) -> i d t", t=NGRP)
            for tl in range(NGRP):
                nc.scalar.activation(
                    y_st3[:, :, tl],
                    ps_y[:, tl * H : (tl + 1) * H],
                    ACT.Copy,
                    scale=rs[:, tl : tl + 1],
                )
            gt0 = c0 + t0
            nc.sync.dma_start(y3[:, :, gt0 : gt0 + NGRP], y_st3)

    # ---------------- Stage 3: projection ----------------
    outp = ctx.enter_context(tc.tile_pool(name=R + "outp", bufs=3))
    with tc.tile_pool(name=R + "ypool", bufs=1) as ypool:
        yt_sb = ypool.tile([128, NK * NTOK], BF16)
        for p2 in range(2):
            nc.sync.dma_start(
                yt_sb.rearrange("(p2 d) (k t) -> p2 d k t", p2=2, t=NTOK)[p2],
                Ydram.rearrange("(k p2) (d t) -> p2 d k t", p2=2, t=NTOK)[p2],
            )
        for mo in range(NK):
            wp = wpool.tile([128, NK * 128], BF16, name=f"{R}wp{mo}", tag="wa")
            nc.sync.dma_start(
                wp.rearrange("p (k o) -> p k o", o=128),
                WpT.ap()[:, mo * 128 : (mo + 1) * 128].rearrange(
                    "(k p) o -> p k o", p=128
                ),
            )
            ps = ps1.tile([128, NTOK], F32, name=f"{R}pso{mo}", tag="ps")
            for kc in range(NK):
                nc.tensor.matmul(
                    ps[:],
                    wp[:, kc * 128 : (kc + 1) * 128],
                    yt_sb[:, kc * NTOK : (kc + 1) * NTOK],
                    start=(kc == 0),
                    stop=(kc == NK - 1),
                )
            ob = outp.tile([128, NTOK], F32, name=f"{R}ob{mo}", tag="ob")
            nc.scalar.activation(
                ob[:], ps[:], ACT.Identity, bias=b_prj_sb[:, mo : mo + 1]
            )
            nc.sync.dma_start(outT.ap()[mo * 128 : (mo + 1) * 128, :], ob[:])


_PROGRAMS = {}


def _get_program(reps=1):
    if reps not in _PROGRAMS:
        _PROGRAMS[reps] = _build_program(reps)
    return _PROGRAMS[reps]


def _host_inputs(x, W_atten, b_atten, W_proj, b_proj):
    bf = ml_dtypes.bfloat16
    x = np.asarray(x, np.float32).reshape(B, T, C)
    WaT = np.ascontiguousarray(np.asarray(W_atten, np.float32).T).astype(bf)
    WpT = np.ascontiguousarray(np.asarray(W_proj, np.float32).T).astype(bf)
    ba = np.asarray(b_atten, np.float32).copy()
    ba[C : 2 * C] *= 0.125  # fold the 1/sqrt(hd) score scale into k bias
    b_att_h = np.ascontiguousarray(ba.reshape(NM, 128).T)
    b_prj_h = np.ascontiguousarray(
        np.asarray(b_proj, np.float32).reshape(NK, 128).T
    )
    mask = np.tril(np.ones((H, H), np.float32))  # keep j<=i
    maskT8_h = np.ascontiguousarray(np.tile(mask.T, (1, NGRP))).astype(bf)
    ones_h = np.ones((H, 1), bf)

    shard_b = B // N_CORES
    in_maps = []
    for i in range(N_CORES):
        xs = x[i * shard_b : (i + 1) * shard_b].reshape(NTOK, C)
        xT_i = np.ascontiguousarray(xs.T).astype(bf)
        in_maps.append(
            {
                "xT": xT_i,
                "WaT": WaT,
                "WpT": WpT,
                "b_att": b_att_h,
                "b_prj": b_prj_h,
                "maskT8": maskT8_h,
                "onesc": ones_h,
            }
        )
    return in_maps


def run(inputs, trace=False):
    nc = _get_program()
    in_maps = _host_inputs(**inputs)
    res = run_bass_kernel_spmd(
        nc, in_maps, list(range(N_CORES)), trace=trace
    )
    shard_b = B // N_CORES
    out = np.empty((B, T, C), dtype=np.float32)
    for i in range(N_CORES):
        oT = np.asarray(res.results[i]["outT"], np.float32)  # (C, NTOK)
        out[i * shard_b : (i + 1) * shard_b] = oT.T.reshape(shard_b, T, C)
    return out, res


def kernel(x, W_atten, b_atten, W_proj, b_proj):
    out, _ = run(
        dict(
            x=x,
            W_atten=W_atten,
            b_atten=b_atten,
            W_proj=W_proj,
            b_proj=b_proj,
        )
    )
    return out
